# revision 1
# baseline (speedup 1.0000x reference)
"""BKT (Bayesian Knowledge Tracing) forward pass on Trainium2, 8 NeuronCores.

The reference's chunked 32-trajectory scan is mathematically a 2-state HMM
forward pass. Per (sequence, t):
    alpha' = alpha @ (diag(o_t) @ Tr)      (row vector times matrix)
with o_s(t) = P(obs_t | L=s), Tr the 2x2 BKT transition matrix, and
    out_c(t) = log(alpha@pc) - log(alpha@1),  pc = [P(c|0), P(c|1)].

Device algorithm (per core, batch-parallel over 2048 sequences laid out as
128 partitions x 16 groups, free dim = (t, g)):
  1. ACT sigmoids give observation probs; the corr-select is folded into the
     sigmoid argument via sign flip: o0 = sigmoid((2c-1)*lg).
  2. Per-step 2x2 matrices W_t, chunk products A_c over K=10 steps built with
     fused stride-0-broadcast tensor_tensor folds (parallel over chunks).
  3. Short serial recursion over chunk matrices -> chunk-start alphas.
  4. Within-chunk recovery (parallel over chunks) -> per-t alphas.
  5. Predictions + log-softmax via Ln(num*s) - Ln(den*s), s = 2^-exp(den)
     (exact power-of-two rescale keeps the ACT Ln LUT in its sane range).

Sharding: pure data-parallel over batch; parameter tables are gathered on
host (traffic-neutral: 8B/element of gathered logits replaces the 8B int64
problem id), all recurrences stay on-device.
"""

import numpy as np

import concourse.bass as bass
import concourse.bacc as bacc
import concourse.tile as tile
import concourse.mybir as mybir
from concourse._compat import with_exitstack

F32 = mybir.dt.float32
U8 = mybir.dt.uint8
AF = mybir.ActivationFunctionType
OP = mybir.AluOpType

P = 128          # partitions
N_CORES = 8


def emit_bkt(nc, G, T, K, SEG, renorm_every=2):
    """Emit the BKT kernel for one core. Sequences = P*G, free layout (t, g).

    Software-pipelined over T-segments: segment s+1's observation sigmoids
    (ACT) are emitted before segment s's Ln calls, and segment s's final
    log-subtract is emitted after segment s+1's W-build, so neither engine
    stalls on the other at segment boundaries.

    DRAM tensors:
      lls:  (P, T, 2, G) f32  packed [guess, slip] logits
      cm:   (P, T, G) i8      2*corr-1 in {-1, +1}
      dyn:  (P, 3, G) f32     [logit_pL, logit_pF, logit_pI0]
      out:  (P, T, 2, G) f32  [log p(incorrect), log p(correct)]
    """
    assert T % SEG == 0 and SEG % K == 0
    NSEG = T // SEG
    CS = SEG // K          # chunks per segment
    CT = T // K            # total chunks

    lls_d = nc.dram_tensor("lls", [P, T, 2, G], F32, kind="ExternalInput")
    cm_d = nc.dram_tensor("cm", [P, T, G], mybir.dt.int8, kind="ExternalInput")
    dyn_d = nc.dram_tensor("dyn", [P, 3, G], F32, kind="ExternalInput")
    out_d = nc.dram_tensor("out", [P, T, 2, G], F32, kind="ExternalOutput")

    with tile.TileContext(nc) as tc:
        with (
            tc.tile_pool(name="singles", bufs=1) as singles,
            tc.tile_pool(name="io", bufs=2) as io,
            tc.tile_pool(name="work", bufs=1) as work,
            tc.tile_pool(name="actb", bufs=2) as actb,
        ):
            # ---- per-sequence constants ----
            dyn_t = singles.tile([P, 3, G], F32)
            nc.sync.dma_start(dyn_t[:], dyn_d[:])
            # Tr packed [s][s']: [[1-l, l], [f, 1-f]]; 1-sigmoid(x) = sigmoid(-x)
            Tp = singles.tile([P, 2, G, 2], F32)   # [s][g][s']
            nc.scalar.activation(Tp[:, 0, :, 0], dyn_t[:, 0, :], AF.Sigmoid, scale=-1.0)
            nc.scalar.activation(Tp[:, 0, :, 1], dyn_t[:, 0, :], AF.Sigmoid)
            nc.scalar.activation(Tp[:, 1, :, 0], dyn_t[:, 1, :], AF.Sigmoid)
            nc.scalar.activation(Tp[:, 1, :, 1], dyn_t[:, 1, :], AF.Sigmoid, scale=-1.0)

            # chunk-start alphas, all chunks + final carry
            starts = singles.tile([P, CT + 1, 2, G], F32)
            nc.scalar.activation(starts[:, 0, 0, :], dyn_t[:, 2, :], AF.Sigmoid, scale=-1.0)
            nc.scalar.activation(starts[:, 0, 1, :], dyn_t[:, 2, :], AF.Sigmoid)

            obs = {}        # per-seg live tiles from phase A
            fin = {}        # per-seg live tiles awaiting finalize

            def phase_a(seg, nsplit=1):
                """Loads + observation sigmoids for segment seg. nsplit > 1
                slices the DMA + sigmoid chain so compute starts on the first
                slice while later slices are still in flight (startup ramp)."""
                s0 = seg * SEG
                lls = io.tile([P, SEG, 2, G], F32, tag="lls")
                cmt = io.tile([P, SEG, G], mybir.dt.int8, tag="cm")
                zpk = work.tile([P, SEG, 2, G], F32, tag="zpk")
                op_t = actb.tile([P, SEG, 2, G], F32, tag="opack")
                ptp = actb.tile([P, SEG, 2, G], F32, tag="ptp")
                bounds = [SEG * h // nsplit for h in range(nsplit + 1)]
                for h in range(nsplit):
                    a, b = bounds[h], bounds[h + 1]
                    nc.sync.dma_start(lls[:, a:b], lls_d[:, s0 + a : s0 + b, :, :])
                    nc.sync.dma_start(cmt[:, a:b], cm_d[:, s0 + a : s0 + b, :])
                    # o_s(t) = sigmoid(+-logit): corr-select via sign flip
                    nc.vector.tensor_tensor(
                        zpk[:, a:b], lls[:, a:b],
                        cmt[:, a:b].unsqueeze(2).broadcast_to((P, b - a, 2, G)),
                        OP.mult,
                    )
                    nc.scalar.activation(op_t[:, a:b, 0, :], zpk[:, a:b, 0, :], AF.Sigmoid)
                    nc.scalar.activation(op_t[:, a:b, 1, :], zpk[:, a:b, 1, :], AF.Sigmoid, scale=-1.0)
                    # true-outcome probs for predictions: [P(c|0), P(c|1)]
                    nc.scalar.activation(ptp[:, a:b, 0, :], lls[:, a:b, 0, :], AF.Sigmoid)
                    nc.scalar.activation(ptp[:, a:b, 1, :], lls[:, a:b, 1, :], AF.Sigmoid, scale=-1.0)
                obs[seg] = (op_t, ptp)

            def finalize(seg):
                """Log-subtract + store for segment seg (after its ACT Lns)."""
                s0 = seg * SEG
                out_t, _ = fin.pop(seg)
                h = SEG // 2
                nc.sync.dma_start(out_d[:, s0 : s0 + h, :, :], out_t[:, :h])
                nc.sync.dma_start(out_d[:, s0 + h : s0 + SEG, :, :], out_t[:, h:])

            def phase_b(seg):
                """W-build, folds, serial recursion, recovery, predictions."""
                c0 = seg * CS
                op_t, ptp = obs.pop(seg)

                # per-step matrices W[t][s][s'][g] = o_s(t) * Tr[s][s']
                Wp = work.tile([P, SEG, 2, G, 2], F32, tag="Wp")   # [t][s][g][s']
                nc.vector.tensor_tensor(
                    Wp[:],
                    op_t[:].unsqueeze(4).broadcast_to((P, SEG, 2, G, 2)),
                    Tp[:].unsqueeze(1).broadcast_to((P, SEG, 2, G, 2)),
                    OP.mult,
                )
                Wc = Wp[:].rearrange("p (c k) s g t -> p c k s g t", k=K)

                if seg >= 1:
                    finalize(seg - 1)

                # chunk products A_c = W_{ck} @ ... @ W_{ck+K-1}
                A = work.tile([P, CS, 2, 2, G], F32, tag="A")      # [c][i][s'][g]
                Ax = A[:].rearrange("p c i s g -> p c i g s")      # iterate (c,i,g,s')
                TM = work.tile([P, CS, 2, 2, G, 2], F32, tag="TM")  # [c][i][m][g][s']
                nc.scalar.copy(Ax, Wc[:, :, 0])
                for j in range(1, K):
                    Wj = Wc[:, :, j]      # (P, CS, 2, G, 2) = [c][m][g][s']
                    # TM[i,m,g,s'] = A[i,m]*W[m,s'] in one op (APs merge <=3D),
                    # then A'[i,s'] = TM[i,0,s'] + TM[i,1,s']
                    nc.vector.tensor_tensor(
                        TM[:],
                        A[:].unsqueeze(5).broadcast_to((P, CS, 2, 2, G, 2)),
                        Wj[:].unsqueeze(2).broadcast_to((P, CS, 2, 2, G, 2)),
                        OP.mult,
                    )
                    nc.vector.tensor_tensor(Ax, TM[:, :, :, 0], TM[:, :, :, 1], OP.add)

                # serial chunk recursion:
                # sv[m][s'] = starts[m]*A[m,s'] ; starts' = sv[0]+sv[1]
                sv = work.tile([P, 2, 2, G], F32, tag="sv")
                ssum = work.tile([P, G], F32, tag="ssum")
                for cl in range(CS):
                    cg = c0 + cl
                    st = starts[:, cg]
                    stn = starts[:, cg + 1]
                    nc.vector.tensor_tensor(
                        sv[:],
                        st[:].unsqueeze(2).broadcast_to((P, 2, 2, G)),
                        A[:, cl],
                        OP.mult,
                    )
                    nc.vector.tensor_tensor(stn, sv[:, 0], sv[:, 1], OP.add)
                    if cg % renorm_every == renorm_every - 1:
                        nc.vector.tensor_tensor(
                            ssum[:], stn[:, 0, :], stn[:, 1, :], OP.add
                        )
                        nc.vector.reciprocal_approx_fast(ssum[:], ssum[:])
                        nc.vector.tensor_tensor(
                            stn,
                            stn,
                            ssum[:].unsqueeze(1).broadcast_to((P, 2, G)),
                            OP.mult,
                        )

                # within-chunk recovery: per-t alphas
                rec = work.tile([P, SEG, 2, G], F32, tag="rec")
                rc = rec[:].rearrange("p (c k) s g -> p c k s g", k=K)
                nc.scalar.copy(rc[:, :, 0], starts[:, c0 : c0 + CS])
                RR = work.tile([P, CS, 2, G, 2], F32, tag="RR")   # [c][m][g][s']
                for j in range(1, K):
                    prev = rc[:, :, j - 1]   # (P, CS, 2, G) = [c][m][g]
                    nc.vector.tensor_tensor(
                        RR[:],
                        prev[:].unsqueeze(4).broadcast_to((P, CS, 2, G, 2)),
                        Wc[:, :, j - 1],
                        OP.mult,
                    )
                    nc.vector.tensor_tensor(
                        rc[:, :, j].rearrange("p c s g -> p c g s"),
                        RR[:, :, 0], RR[:, :, 1], OP.add,
                    )

                # predictions; the last segment runs in halves so its Ln +
                # store overlap the second half's vector work (tail exposure)
                qp = work.tile([P, SEG, 2, G], F32, tag="qp")
                pn = work.tile([P, SEG, 2, G], F32, tag="pn")
                den = work.tile([P, SEG, G], F32, tag="den")
                rr = work.tile([P, SEG, G], F32, tag="rr")
                out_t = io.tile([P, SEG, 2, G], F32, tag="out")
                nsp = 2 if seg == NSEG - 1 else 1
                bounds = [SEG * h // nsp for h in range(nsp + 1)]
                for hh in range(nsp):
                    a, b = bounds[hh], bounds[hh + 1]
                    n = b - a
                    nc.vector.tensor_tensor(qp[:, a:b], rec[:, a:b], ptp[:, a:b], OP.mult)
                    # pn[t][1] = num (correct mass), pn[t][0] = den - num
                    nc.vector.tensor_tensor(
                        pn[:, a:b, 1, :], qp[:, a:b, 0, :], qp[:, a:b, 1, :], OP.add
                    )
                    nc.vector.tensor_tensor(
                        den[:, a:b], rec[:, a:b, 0, :], rec[:, a:b, 1, :], OP.add
                    )
                    nc.vector.tensor_tensor(
                        pn[:, a:b, 0, :], den[:, a:b], pn[:, a:b, 1, :], OP.subtract
                    )
                    # Normalize by r ~= 1/den (~51 ULP): out = Ln(pn*r). The
                    # approximation error shifts both outputs by -Ln(den*r)
                    # ~ 4e-6 (harmless), avoids the Ln LUT's bad range below
                    # ~2^-50, and replaces the exponent-rescale pipeline.
                    nc.vector.reciprocal_approx_fast(rr[:, a:b], den[:, a:b])
                    nc.vector.tensor_tensor(
                        pn[:, a:b], pn[:, a:b],
                        rr[:, a:b].unsqueeze(2).broadcast_to((P, n, 2, G)), OP.mult,
                    )
                    m = (a + b) // 2
                    nc.scalar.activation(out_t[:, a:m], pn[:, a:m], AF.Ln)
                    nc.scalar.activation(out_t[:, m:b], pn[:, m:b], AF.Ln)
                fin[seg] = (out_t, None)

            for seg in range(NSEG):
                phase_a(seg, nsplit=(4 if seg == 0 else 1))
                if seg >= 1:
                    phase_b(seg - 1)
            phase_b(NSEG - 1)
            finalize(NSEG - 1)

    return nc


# ------------------------------------------------------------------
# Host-side full-problem wrapper
# ------------------------------------------------------------------

_B, _T, _K, _SEG = 16384, 500, 10, 100
_G = _B // (P * N_CORES)   # 16 groups per core

_cached = {}


def _build():
    if "nc" not in _cached:
        nc = bacc.Bacc(None, target_bir_lowering=False)
        emit_bkt(nc, G=_G, T=_T, K=_K, SEG=_SEG)
        nc.compile()
        _cached["nc"] = nc
    return _cached["nc"]


def _shard(arr, core):
    """(B,...) -> this core's (P, ..., G) permuted view, seq = g*128 + p."""
    rows = arr[core * P * _G : (core + 1) * P * _G]
    r = rows.reshape(_G, P, *arr.shape[1:])
    order = (1,) + tuple(range(2, r.ndim)) + (0,)
    return np.ascontiguousarray(r.transpose(order))


def kernel(corr, kc, problem, dynamics_logits_table, obs_logits_kc,
           obs_logits_problem, fastbkt_n):
    from concourse.bass_utils import run_bass_kernel_spmd

    corr = np.asarray(corr, dtype=np.float32)
    kc = np.asarray(kc).astype(np.int64)
    problem = np.asarray(problem).astype(np.int64)
    dyn_table = np.asarray(dynamics_logits_table, dtype=np.float32)
    obs_kc = np.asarray(obs_logits_kc, dtype=np.float32)
    obs_prob = np.asarray(obs_logits_problem, dtype=np.float32)

    B, T = corr.shape
    assert B == _B and T == _T, (B, T)

    # host gathers (traffic-neutral input marshaling)
    lls = obs_kc[kc][:, None, :] + obs_prob[problem]       # (B, T, 2)
    dyn = dyn_table[kc]                                    # (B, 3)
    cm8 = (corr * 2.0 - 1.0).astype(np.int8)

    nc = _build()
    in_maps = []
    for core in range(N_CORES):
        in_maps.append({
            "lls": _shard(lls, core),
            "cm": _shard(cm8, core),
            "dyn": _shard(dyn, core),
        })

    res = run_bass_kernel_spmd(
        nc, in_maps, core_ids=list(range(N_CORES)), **_cached.get("run_kwargs", {})
    )
    _cached["last_results"] = res

    out = np.empty((B, T, 2), np.float32)
    for core in range(N_CORES):
        o = res.results[core]["out"]                       # (P, T, 2, G)
        rows = o.transpose(3, 0, 1, 2).reshape(P * _G, T, 2)
        out[core * P * _G : (core + 1) * P * _G] = rows
    return out



# revision 10
# speedup vs baseline: 1.6821x; 1.6821x over previous
"""BKT (Bayesian Knowledge Tracing) forward pass on Trainium2, 8 NeuronCores.

The reference's chunked 32-trajectory scan is a 2-state HMM forward pass.
Per (sequence, t):  alpha' = alpha @ (diag(o_t) @ Tr), with o_s(t) =
P(obs_t | L=s) and Tr the 2x2 BKT transition matrix. The output is the
log-softmax over [P(incorrect), P(correct)], i.e. per-t it only depends on
the normalized alpha — every intermediate may carry an arbitrary per-t scale,
which this kernel exploits aggressively.

v3 design (engine-balanced, fp16 2x-mode core, software-pipelined 2 deep):
  - Host sends sign-flipped logits zpk (fp16) so one ACT Sigmoid call gives
    o_s(t) = P(observed outcome | s); the device emits [log(1-r), log r] with
    r = P(observed)/den and the host swaps slots where corr==0 (marshaling).
  - Per-step matrices W = o x (2*Tr) in fp16. Chunk products = two half-chunk
    products of 5 (fp16, range-safe [2^-15, 2^4]), joined in f32 on Pool.
  - Chunk matrices are sum-normalized (one DVE reciprocal per segment), which
    keeps the 50-step f32 serial chunk recursion on Pool bounded (max drift
    2^79 on this data) with NO in-loop renormalization or division.
  - Within-chunk recovery in fp16 from reciprocal-normalized chunk starts,
    restarting mid-chunk (per-chunk/per-half scales cancel in r).
  - Predictions: qp in f32 (DVE), pair-sums on Pool, three ACT Ln calls,
    final log-softmax subtractions on DVE in fp16.
  - Pipeline skew: segment s's fold (DVE) overlaps segment s-1's back half;
    Pool stream ordered [join(s) | preds(s-1) | serial(s)] so the in-order
    Pool queue never blocks on a not-yet-ready stage.
All hot-loop DVE traffic is 2-byte packed (0.52 ns/elem 2x mode); DRAM
arrays are host-packed so every DMA descriptor is a 6.4KB contiguous run.

Sharding: pure data-parallel over batch (2048 sequences/core as 128
partitions x 16 groups); parameter tables gathered on host.
"""

import numpy as np

import concourse.bass as bass
import concourse.bacc as bacc
import concourse.tile as tile
import concourse.mybir as mybir

F32 = mybir.dt.float32
F16 = mybir.dt.float16
AF = mybir.ActivationFunctionType
OP = mybir.AluOpType

P = 128          # partitions
N_CORES = 8
GAMMA = 2.0      # per-step scale baked into Tr: keeps fp16 products ~1


def emit_bkt(nc, G, T, K, SEG):
    """Emit the BKT kernel for one core. Sequences = P*G, free layout (.., g).

    DRAM tensors:
      zpk:  (P, T, 2, G) f16  sign-flipped [guess, slip] logits:
            zpk[..0] = (2c-1)*lg, zpk[..1] = -(2c-1)*ls
      dyn:  (P, 3, G) f32     [logit_pL, logit_pF, logit_pI0]
      out:  (P, T, 2, G) f16  [log(1-r), log r], r = P(observed outcome)
    """
    assert T % SEG == 0 and SEG % K == 0 and K % 2 == 0
    NSEG = T // SEG
    CS = SEG // K          # chunks per segment
    CT = T // K            # total chunks
    H = K // 2             # half-chunk length
    C2 = 2 * CS            # half-chunks per segment

    zpk_d = nc.dram_tensor("zpk", [P, T, 2, G], F16, kind="ExternalInput")
    dyn_d = nc.dram_tensor("dyn", [P, 3, G], F32, kind="ExternalInput")
    out_d = nc.dram_tensor("out", [P, T, 2, G], F16, kind="ExternalOutput")

    with tile.TileContext(nc) as tc:
        with (
            tc.tile_pool(name="singles", bufs=1) as singles,
            tc.tile_pool(name="dbuf", bufs=2) as dbuf,
            tc.tile_pool(name="scratch", bufs=1) as scratch,
        ):
            # ---- per-sequence constants ----
            dyn_t = singles.tile([P, 3, G], F32)
            nc.sync.dma_start(dyn_t[:], dyn_d[:])
            Ttmp = singles.tile([P, 2, 2, G], F32)   # Tr[s][s'][g]
            nc.scalar.activation(Ttmp[:, 0, 0], dyn_t[:, 0], AF.Sigmoid, scale=-1.0)
            nc.scalar.activation(Ttmp[:, 0, 1], dyn_t[:, 0], AF.Sigmoid)
            nc.scalar.activation(Ttmp[:, 1, 0], dyn_t[:, 1], AF.Sigmoid)
            nc.scalar.activation(Ttmp[:, 1, 1], dyn_t[:, 1], AF.Sigmoid, scale=-1.0)
            Tp = singles.tile([P, 2, 2, G], F16)     # gamma * Tr
            nc.scalar.mul(Tp[:], Ttmp[:], GAMMA)

            # chunk-start alphas (f32), all chunks + final carry
            starts = singles.tile([P, CT + 1, 2, G], F32)
            nc.scalar.activation(starts[:, 0, 0], dyn_t[:, 2], AF.Sigmoid, scale=-1.0)
            nc.scalar.activation(starts[:, 0, 1], dyn_t[:, 2], AF.Sigmoid)

            obs = {}       # seg -> op tile (sigmoid outputs)
            mats = {}      # seg -> (Wp, Ah) tiles live into the back half
            fin = {}       # seg -> out tile awaiting store

            def phase_a(seg):
                s0 = seg * SEG
                zpk = dbuf.tile([P, SEG, 2, G], F16, tag="zpk")
                nc.sync.dma_start(zpk[:], zpk_d[:, s0 : s0 + SEG])
                op_t = dbuf.tile([P, SEG, 2, G], F16, tag="op")
                nc.scalar.activation(op_t[:], zpk[:], AF.Sigmoid)
                obs[seg] = op_t

            def front(seg):
                """W build + half-chunk fold (DVE) + f32 join (Pool) +
                A-normalization (DVE) staged for the Pool serial chain."""
                # W[t][s][s'][g] = o[t][s][g] * (gamma Tr)[s][s'][g]   (fp16)
                op_t = obs[seg]
                Wp = dbuf.tile([P, SEG, 2, 2, G], F16, tag="Wp")
                for s in range(2):   # split keeps reads within 3 AP dims
                    nc.vector.tensor_tensor(
                        Wp[:, :, s],
                        op_t[:, :, s].unsqueeze(2).broadcast_to((P, SEG, 2, G)),
                        Tp[:, s].unsqueeze(1).broadcast_to((P, SEG, 2, G)),
                        OP.mult,
                    )
                Wh = Wp[:].rearrange("p (c h) s u g -> p c h s u g", h=H)

                # half-chunk products Ah[c2][i][s'][g] (fp16)
                Ah = dbuf.tile([P, C2, 2, 2, G], F16, tag="Ah")
                nc.scalar.copy(Ah[:], Wh[:, :, 0])
                TMh = scratch.tile([P, C2, 2, 2, 2, G], F16, tag="TMh")
                for j in range(1, H):
                    nc.vector.tensor_tensor(
                        TMh[:],
                        Ah[:].unsqueeze(4).broadcast_to((P, C2, 2, 2, 2, G)),
                        Wh[:, :, j].unsqueeze(2).broadcast_to((P, C2, 2, 2, 2, G)),
                        OP.mult,
                    )
                    nc.vector.tensor_tensor(
                        Ah[:], TMh[:, :, :, 0], TMh[:, :, :, 1], OP.add
                    )
                mats[seg] = (Wp, Ah)

                # join halves -> full chunk products A (f32) on Pool
                Ahv = Ah[:].rearrange("p (c h) i u g -> p c h i u g", h=2)
                TM2 = scratch.tile([P, CS, 2, 2, 2, G], F32, tag="TM2")
                for i in range(2):   # split keeps reads within 3 AP dims
                    for m in range(2):
                        nc.gpsimd.tensor_tensor(
                            TM2[:, :, i, m],
                            Ahv[:, :, 0, i, m].unsqueeze(2).broadcast_to(
                                (P, CS, 2, G)),
                            Ahv[:, :, 1, m],
                            OP.mult,
                        )
                A = scratch.tile([P, CS, 2, 2, G], F32, tag="A")
                nc.gpsimd.tensor_tensor(
                    A[:], TM2[:, :, :, 0], TM2[:, :, :, 1], OP.add
                )
                return A

            def a_norm(seg, A):
                """Sum-normalize chunk matrices (DVE) so the serial chain
                needs no in-loop renorm; any per-chunk scale cancels."""
                uA = scratch.tile([P, CS, 2, G], F32, tag="uA")
                nc.vector.tensor_tensor(uA[:], A[:, :, 0], A[:, :, 1], OP.add)
                tA = scratch.tile([P, CS, G], F32, tag="tA")
                nc.vector.tensor_tensor(tA[:], uA[:, :, 0], uA[:, :, 1], OP.add)
                nc.vector.reciprocal_approx_fast(tA[:], tA[:])
                Af = A[:].rearrange("p c i u g -> p c (i u) g")
                nc.vector.tensor_tensor(
                    Af, Af,
                    tA[:].unsqueeze(2).broadcast_to((P, CS, 4, G)),
                    OP.mult,
                )

            def serial(seg, A):
                """50-step chunk recursion on Pool, f32, no renorm."""
                c0 = seg * CS
                sv = scratch.tile([P, 2, 2, G], F32, tag="sv")
                for cl in range(CS):
                    cg = c0 + cl
                    nc.gpsimd.tensor_tensor(
                        sv[:],
                        starts[:, cg].unsqueeze(2).broadcast_to((P, 2, 2, G)),
                        A[:, cl],
                        OP.mult,
                    )
                    nc.gpsimd.tensor_tensor(
                        starts[:, cg + 1], sv[:, 0], sv[:, 1], OP.add
                    )

            def back(seg):
                """Recovery + predictions for segment seg."""
                c0 = seg * CS
                op_t = obs.pop(seg)
                Wp, Ah = mats.pop(seg)
                Wc = Wp[:].rearrange("p (c k) s u g -> p c k s u g", k=K)
                Ahv = Ah[:].rearrange("p (c h) i u g -> p c h i u g", h=2)

                # normalized fp16 chunk starts -> rec[.,.,0]
                rec = dbuf.tile([P, CS, K, 2, G], F16, tag="rec")
                stseg = starts[:, c0 : c0 + CS]
                ssc = scratch.tile([P, CS, G], F32, tag="ssc")
                nc.vector.tensor_tensor(
                    ssc[:], stseg[:, :, 0], stseg[:, :, 1], OP.add
                )
                nc.vector.reciprocal_approx_fast(ssc[:], ssc[:])
                nc.vector.tensor_tensor(
                    rec[:, :, 0], stseg,
                    ssc[:].unsqueeze(2).broadcast_to((P, CS, 2, G)),
                    OP.mult,
                )

                # mid-chunk restart: S5 = stn16 . Ah_even, renormalized
                TM5 = scratch.tile([P, CS, 2, 2, G], F16, tag="TM5")
                for i in range(2):
                    nc.vector.tensor_tensor(
                        TM5[:, :, i],
                        rec[:, :, 0, i].unsqueeze(2).broadcast_to((P, CS, 2, G)),
                        Ahv[:, :, 0, i],
                        OP.mult,
                    )
                S5 = scratch.tile([P, CS, 2, G], F16, tag="S5")
                nc.vector.tensor_tensor(S5[:], TM5[:, :, 0], TM5[:, :, 1], OP.add)
                ss5 = scratch.tile([P, CS, G], F32, tag="ss5")
                nc.vector.tensor_tensor(ss5[:], S5[:, :, 0], S5[:, :, 1], OP.add)
                nc.vector.reciprocal_approx_fast(ss5[:], ss5[:])
                nc.vector.tensor_tensor(
                    rec[:, :, H], S5[:],
                    ss5[:].unsqueeze(2).broadcast_to((P, CS, 2, G)),
                    OP.mult,
                )

                # within-chunk recovery (fp16), both halves
                RR = scratch.tile([P, CS, 2, 2, G], F16, tag="RR")
                for j in list(range(1, H)) + list(range(H + 1, K)):
                    for m in range(2):
                        nc.vector.tensor_tensor(
                            RR[:, :, m],
                            rec[:, :, j - 1, m].unsqueeze(2).broadcast_to(
                                (P, CS, 2, G)),
                            Wc[:, :, j - 1, m],
                            OP.mult,
                        )
                    nc.vector.tensor_tensor(
                        rec[:, :, j], RR[:, :, 0], RR[:, :, 1], OP.add
                    )

                # predictions
                rec_f = rec[:].rearrange("p c k s g -> p (c k) s g")
                qp = scratch.tile([P, SEG, 2, G], F32, tag="qp")
                nc.vector.tensor_tensor(qp[:], rec_f, op_t[:], OP.mult)
                pred = scratch.tile([P, SEG, G], F32, tag="pred")
                den = scratch.tile([P, SEG, G], F32, tag="den")
                predi = scratch.tile([P, SEG, G], F32, tag="predi")
                nc.gpsimd.tensor_tensor(pred[:], qp[:, :, 0], qp[:, :, 1], OP.add)
                nc.gpsimd.tensor_tensor(
                    den[:], rec_f[:, :, 0], rec_f[:, :, 1], OP.add
                )
                nc.gpsimd.tensor_tensor(predi[:], den[:], pred[:], OP.subtract)
                lnp = scratch.tile([P, SEG, G], F32, tag="lnp")
                lni = scratch.tile([P, SEG, G], F32, tag="lni")
                lnd = scratch.tile([P, SEG, G], F32, tag="lnd")
                nc.scalar.activation(lnp[:], pred[:], AF.Ln)
                nc.scalar.activation(lni[:], predi[:], AF.Ln)
                nc.scalar.activation(lnd[:], den[:], AF.Ln)
                out_t = dbuf.tile([P, SEG, 2, G], F16, tag="out")
                nc.vector.tensor_tensor(out_t[:, :, 1], lnp[:], lnd[:], OP.subtract)
                nc.vector.tensor_tensor(out_t[:, :, 0], lni[:], lnd[:], OP.subtract)
                fin[seg] = out_t

            def finalize(seg):
                s0 = seg * SEG
                out_t = fin.pop(seg)
                nc.sync.dma_start(out_d[:, s0 : s0 + SEG], out_t[:])

            # ---- software pipeline, 2 segments deep ----
            phase_a(0)
            phase_a(1)
            for seg in range(NSEG):
                A = front(seg)
                if seg >= 1:
                    back(seg - 1)
                a_norm(seg, A)
                serial(seg, A)
                if seg >= 1:
                    finalize(seg - 1)
                if seg + 2 < NSEG:
                    phase_a(seg + 2)
            back(NSEG - 1)
            finalize(NSEG - 1)

    return nc


# ------------------------------------------------------------------
# Host-side full-problem wrapper
# ------------------------------------------------------------------

_B, _T, _K, _SEG = 16384, 500, 10, 100
_G = _B // (P * N_CORES)   # 16 groups per core

_cached = {}


def _build():
    if "nc" not in _cached:
        nc = bacc.Bacc(None, target_bir_lowering=False)
        emit_bkt(nc, G=_G, T=_T, K=_K, SEG=_SEG)
        nc.compile()
        _cached["nc"] = nc
    return _cached["nc"]


def _shard(arr, core):
    """(B,...) -> this core's (P, ..., G) permuted view, seq = g*128 + p."""
    rows = arr[core * P * _G : (core + 1) * P * _G]
    r = rows.reshape(_G, P, *arr.shape[1:])
    order = (1,) + tuple(range(2, r.ndim)) + (0,)
    return np.ascontiguousarray(r.transpose(order))


def kernel(corr, kc, problem, dynamics_logits_table, obs_logits_kc,
           obs_logits_problem, fastbkt_n):
    from concourse.bass_utils import run_bass_kernel_spmd

    corr = np.asarray(corr, dtype=np.float32)
    kc = np.asarray(kc).astype(np.int64)
    problem = np.asarray(problem).astype(np.int64)
    dyn_table = np.asarray(dynamics_logits_table, dtype=np.float32)
    obs_kc = np.asarray(obs_logits_kc, dtype=np.float32)
    obs_prob = np.asarray(obs_logits_problem, dtype=np.float32)

    B, T = corr.shape
    assert B == _B and T == _T, (B, T)

    # host gathers + sign-flip (traffic-neutral input marshaling)
    lls = obs_kc[kc][:, None, :] + obs_prob[problem]       # (B, T, 2)
    sgn = (corr * 2.0 - 1.0).astype(np.float32)            # (B, T)
    zpk = np.empty((B, T, 2), np.float16)
    zpk[:, :, 0] = sgn * lls[:, :, 0]
    zpk[:, :, 1] = -sgn * lls[:, :, 1]
    dyn = dyn_table[kc]                                    # (B, 3)

    nc = _build()
    in_maps = []
    for core in range(N_CORES):
        in_maps.append({
            "zpk": _shard(zpk, core),
            "dyn": _shard(dyn, core),
        })

    res = run_bass_kernel_spmd(
        nc, in_maps, core_ids=list(range(N_CORES)), **_cached.get("run_kwargs", {})
    )
    _cached["last_results"] = res

    # unshard + slot swap: device slot1 = log P(observed), slot0 = log P(other)
    dev = np.empty((B, T, 2), np.float32)
    for core in range(N_CORES):
        o = res.results[core]["out"].astype(np.float32)    # (P, T, 2, G)
        rows = o.transpose(3, 0, 1, 2).reshape(P * _G, T, 2)
        dev[core * P * _G : (core + 1) * P * _G] = rows
    c1 = corr > 0.5
    out = np.empty((B, T, 2), np.float32)
    out[:, :, 1] = np.where(c1, dev[:, :, 1], dev[:, :, 0])
    out[:, :, 0] = np.where(c1, dev[:, :, 0], dev[:, :, 1])
    return out
